# revision 1
# baseline (speedup 1.0000x reference)
"""Multi-head causal attention (B=4, S=2048, D=1024, H=16) on 8 TRN2 cores.

Sharding: data-parallel over batch (4) x tensor-parallel over heads (2 groups
of 8 heads). Core c handles batch c//2, head-group c%2. Each core computes
q/k/v projections for its 8 heads, causal flash-style attention, and a partial
output projection against its row-shard of Wp. Host sums the two partials per
batch and adds the bias terms (bp + bv @ Wp, which commute with the row-sum).

Kernel layout: activations kept transposed (qT/kT = [head*e, seq]); scores are
computed as S^T [k, q] so the softmax denominator falls out of the AV matmul
via a ones-column appended to V (flash-attention trick), and causality is
handled block-wise: fully-masked 128-key blocks are skipped, diagonal blocks
are multiplied by precomputed mask slices.
"""

import numpy as np

B, S, D, H = 4, 2048, 1024, 16
HD = D // H            # 64
HPC = 8                # heads per core
LCOL = HPC * HD        # 512 local columns
NSG = 4                # seq groups of 512
SG = S // NSG          # 512
NKB = S // 128         # 16 key blocks of 128

_CACHE = {}


def _build(cdt_name="bfloat16", debug_dump=False, recip_mode="dma"):
    import concourse.bass as bass
    import concourse.tile as tile
    from concourse import bacc, mybir
    from concourse.masks import make_identity

    f32 = mybir.dt.float32
    cdt = getattr(mybir.dt, cdt_name)

    nc = bacc.Bacc("TRN2", target_bir_lowering=False, debug=False)

    x_d = nc.dram_tensor("x", [S, D], f32, kind="ExternalInput")
    wq_d = nc.dram_tensor("wq", [D, LCOL], f32, kind="ExternalInput")
    wk_d = nc.dram_tensor("wk", [D, LCOL], f32, kind="ExternalInput")
    wv_d = nc.dram_tensor("wv", [D, LCOL], f32, kind="ExternalInput")
    bq_d = nc.dram_tensor("bq", [LCOL], f32, kind="ExternalInput")
    bk_d = nc.dram_tensor("bk", [LCOL], f32, kind="ExternalInput")
    wp_d = nc.dram_tensor("wp", [LCOL, D], f32, kind="ExternalInput")
    mask_d = nc.dram_tensor("mask", [128, 2, 1024], f32, kind="ExternalInput")
    out_d = nc.dram_tensor("out", [S, D], f32, kind="ExternalOutput")

    Exp = mybir.ActivationFunctionType.Exp
    Ident = mybir.ActivationFunctionType.Identity

    with tile.TileContext(nc) as tc:
        with (
            tc.tile_pool(name="consts", bufs=1) as consts,
            tc.tile_pool(name="wstage", bufs=1) as wstage,
            tc.tile_pool(name="xp", bufs=2) as xp,
            tc.tile_pool(name="xtp", bufs=1) as xtp,
            tc.tile_pool(name="acts", bufs=1) as acts,
            tc.tile_pool(name="pp", bufs=6) as pp,
            tc.tile_pool(name="recp", bufs=2) as recp,
            tc.tile_pool(name="orp", bufs=3) as orp,
            tc.tile_pool(name="rp", bufs=2) as rp,
            tc.tile_pool(name="outp", bufs=4) as outp,
            tc.tile_pool(name="drp", bufs=2, space="DRAM") as drp,
            tc.tile_pool(name="ps_a", bufs=2, space="PSUM") as ps_a,
            tc.tile_pool(name="ps_s", bufs=2, space="PSUM") as ps_s,
            tc.tile_pool(name="ps_o", bufs=2, space="PSUM") as ps_o,
        ):
            # ---- constants / weights -------------------------------------
            ident = consts.tile([128, 128], cdt)
            make_identity(nc, ident)

            def load_cast(dram_ap, shape, name):
                stg = wstage.tile(list(shape), f32, name="stg", tag="stg")
                nc.sync.dma_start(out=stg, in_=dram_ap)
                t = consts.tile(list(shape), cdt, name=name)
                nc.vector.tensor_copy(t, stg)
                return t

            wq_sb = load_cast(wq_d.ap().rearrange("(c p) n -> p c n", p=128),
                              (128, 8, LCOL), "wq_sb")
            wk_sb = load_cast(wk_d.ap().rearrange("(c p) n -> p c n", p=128),
                              (128, 8, LCOL), "wk_sb")
            wv_sb = load_cast(wv_d.ap().rearrange("(c p) n -> p c n", p=128),
                              (128, 8, LCOL), "wv_sb")
            mask_sb = load_cast(mask_d.ap(), (128, 2, 1024), "mask_sb")
            wp_sb = load_cast(wp_d.ap().rearrange("(c p) n -> p c n", p=128),
                              (128, 4, D), "wp_sb")

            bq_sb = consts.tile([128, 4], f32)
            nc.sync.dma_start(out=bq_sb, in_=bq_d.ap().rearrange("(c p) -> p c", p=128))
            bk_sb = consts.tile([128, 4], f32)
            nc.sync.dma_start(out=bk_sb, in_=bk_d.ap().rearrange("(c p) -> p c", p=128))

            # ---- persistent activations ----------------------------------
            qT = acts.tile([128, 4, S], cdt)       # [col%128, colblk, seq]
            kT = acts.tile([128, 4, S], cdt)
            v_ext = acts.tile([128, NKB, HPC * (HD + 1)], cdt)  # per kb: 8*(64+1)
            oT = acts.tile([128, 4, S], cdt)
            for l in range(HPC):                   # ones columns for denominator
                nc.vector.memset(v_ext[:, :, 65 * l + 64: 65 * l + 65], 1.0)

            def phase1_gen(g):
                """Generator: yields between chunks so qkv/transpose work can
                be interleaved into the exp-bound attention stretches."""
                x_stg = xp.tile([128, 4, D], f32, name="x_stg", tag="x_stg")
                nc.sync.dma_start(
                    out=x_stg,
                    in_=x_d.ap()[g * SG:(g + 1) * SG, :]
                    .rearrange("(c p) d -> p c d", p=128))
                xc = xp.tile([128, 4, D], cdt, name="xc", tag="xc")
                nc.vector.tensor_copy(xc, x_stg)
                yield
                xT = xtp.tile([128, 8, SG], cdt, name="xT", tag="xT")
                for dc in range(8):
                    pt = ps_a.tile([128, SG], cdt, name="pt", tag="ps_a")
                    for s in range(4):
                        nc.tensor.transpose(
                            pt[:, 128 * s:128 * (s + 1)],
                            xc[:, s, 128 * dc:128 * (dc + 1)], ident)
                    nc.vector.tensor_copy(xT[:, dc, :], pt)
                    yield
                # qT / kT for this seq group
                for w_sb, b_sb, dstT in ((wq_sb, bq_sb, qT), (wk_sb, bk_sb, kT)):
                    for m in range(4):
                        pq = ps_a.tile([128, SG], f32, name="pq", tag="ps_a")
                        for dc in range(8):
                            nc.tensor.matmul(
                                pq, lhsT=w_sb[:, dc, 128 * m:128 * (m + 1)],
                                rhs=xT[:, dc, :], start=(dc == 0), stop=(dc == 7))
                        nc.vector.tensor_scalar_add(
                            dstT[:, m, g * SG:(g + 1) * SG], pq,
                            b_sb[:, m:m + 1])
                        yield
                # v (natural layout) for this seq group
                for s in range(4):
                    pv = ps_a.tile([128, LCOL], f32, name="pv", tag="ps_a")
                    for dc in range(8):
                        nc.tensor.matmul(
                            pv, lhsT=xT[:, dc, 128 * s:128 * (s + 1)],
                            rhs=wv_sb[:, dc, :], start=(dc == 0), stop=(dc == 7))
                    kb = 4 * g + s
                    nc.vector.tensor_copy(
                        v_ext[:, kb, :].rearrange("p (h e) -> p h e", e=65)[:, :, 0:64],
                        pv.rearrange("p (h e) -> p h e", e=64))
                    yield

            def attn(h, qg, tick=lambda: None, queue=None):
                """Emits one (head, q-group) attention unit. exp/AV emission
                lags S emission via `queue` (shared across units so the next
                unit's S matmuls fill the previous unit's pipeline drain)."""
                po = 64 * (h % 2)
                ct = h // 2
                nkb = 4 * qg + 4
                po_sl = slice(po, po + 64)
                q_sl = slice(qg * SG, (qg + 1) * SG)
                psum_o = ps_o.tile([65, SG], f32, name="psum_o", tag="ps_o")

                def exp_av(pi, psum_s):
                    p_sb = pp.tile([128, 2, SG], cdt, name="p_sb", tag="p_sb")
                    nc.scalar.activation(p_sb, psum_s, Exp, scale=0.125)
                    if pi >= 2 * qg:  # diagonal pair: causal mask
                        nc.vector.tensor_mul(
                            p_sb, p_sb,
                            mask_sb[:, pi - 2 * qg, :]
                            .rearrange("p (j q) -> p j q", j=2))
                    for j in range(2):
                        kb = 2 * pi + j
                        nc.tensor.matmul(
                            psum_o, lhsT=v_ext[:, kb, 65 * h:65 * h + 65],
                            rhs=p_sb[:, j, :], start=(kb == 0),
                            stop=(kb == nkb - 1))

                for pi in range(nkb // 2):
                    psum_s = ps_s.tile([128, 2, SG], f32, name="psum_s",
                                       tag="ps_s")
                    for j in range(2):
                        kb = 2 * pi + j
                        nc.tensor.matmul(
                            psum_s[:, j, :],
                            lhsT=kT[po_sl, ct, 128 * kb:128 * (kb + 1)],
                            rhs=qT[po_sl, ct, q_sl], start=True, stop=True)
                        tick()
                    queue.append((exp_av, pi, psum_s))
                    while len(queue) > 2:
                        fn, a, b = queue.pop(0)
                        fn(a, b)
                queue.append(
                    (lambda _a, _b: attn_normalize(h, qg, psum_o), None, None))

            def attn_normalize(h, qg, psum_o):
                po = 64 * (h % 2)
                ct = h // 2
                po_sl = slice(po, po + 64)
                q_sl = slice(qg * SG, (qg + 1) * SG)
                # Stage the AV result to SBUF right away so the PSUM bank
                # frees after one DVE op; the normalization chain then runs
                # entirely off the critical path.
                o_raw = orp.tile([65, SG], f32, name="o_raw", tag="o_raw")
                nc.vector.tensor_copy(o_raw, psum_o)
                r_sb = rp.tile([64, SG], f32, name="r_sb", tag="r_sb")
                if recip_mode == "plain":
                    rec = recp.tile([1, SG], f32, name="rec", tag="rec")
                    nc.vector.reciprocal(rec, o_raw[64:65, :])
                    nc.gpsimd.partition_broadcast(r_sb, rec)
                else:
                    # Round-trip the 512 denominators through DRAM to spread
                    # them over 128 lanes (fast reciprocal), then broadcast
                    # back via a partition-step-0 DRAM read. All DMA latency,
                    # nearly zero engine busy.
                    d1 = drp.tile([1, SG], f32, name="d1", tag="d1")
                    nc.sync.dma_start(out=d1, in_=o_raw[64:65, :])
                    den_t = recp.tile([128, SG // 128], f32, name="den_t",
                                      tag="den_t")
                    nc.sync.dma_start(
                        out=den_t,
                        in_=d1.rearrange("a (c p) -> (a p) c", p=128))
                    rec_t = recp.tile([128, SG // 128], f32, name="rec_t",
                                      tag="rec_t")
                    nc.vector.reciprocal(rec_t, den_t)
                    d2 = drp.tile([1, SG], f32, name="d2", tag="d2")
                    nc.sync.dma_start(
                        out=d2.rearrange("a (c p) -> (a p) c", p=128),
                        in_=rec_t)
                    nc.sync.dma_start(
                        out=r_sb,
                        in_=bass.AP(tensor=d2.tensor, offset=d2.offset,
                                    ap=[[0, 64]] + [list(p) for p in d2.ap[1:]]))
                nc.vector.tensor_mul(oT[po_sl, ct, q_sl], o_raw[0:64, :], r_sb)

            def proj_gen(g):
                for s in range(4):
                    sb = 4 * g + s
                    for j in range(2):
                        ppr = ps_a.tile([128, SG], f32, name="ppr", tag="ps_a")
                        for c in range(4):
                            nc.tensor.matmul(
                                ppr, lhsT=oT[:, c, 128 * sb:128 * (sb + 1)],
                                rhs=wp_sb[:, c, j * SG:(j + 1) * SG],
                                start=(c == 0), stop=(c == 3))
                        o_sb = outp.tile([128, SG], f32, name="o_sb", tag="o_sb")
                        nc.scalar.copy(o_sb, ppr)
                        nc.sync.dma_start(
                            out=out_d.ap()[128 * sb:128 * (sb + 1),
                                           j * SG:(j + 1) * SG],
                            in_=o_sb)
                        yield

            # Emission interleaves last-group projections and next-group
            # qkv/transposes into the exp-bound attention stretches (one fill
            # chunk every few key-blocks) so the tensor engine stays fed.
            for _ in phase1_gen(0):
                pass
            for g in range(NSG):
                fill = []
                if g > 0:
                    fill.append(proj_gen(g - 1))
                if g < NSG - 1:
                    fill.append(phase1_gen(g + 1))
                n_chunks = (8 if g > 0 else 0) + (21 if g < NSG - 1 else 0)
                n_ticks = HPC * (4 * g + 4)
                stride = max(1, n_ticks // max(n_chunks, 1))
                state = {"i": 0}

                def tick():
                    state["i"] += 1
                    if state["i"] % stride == 0 and fill:
                        try:
                            next(fill[0])
                        except StopIteration:
                            fill.pop(0)

                queue = []
                for h in range(HPC):
                    attn(h, g, tick, queue)
                while queue:  # group boundary: drain before proj fill reads oT
                    fn, a, b = queue.pop(0)
                    fn(a, b)
                for gen in fill:  # drain any remaining chunks
                    for _ in gen:
                        pass
            for _ in proj_gen(NSG - 1):
                pass

            if debug_dump:
                for nm, t in (("qT", qT), ("kT", kT), ("v_ext", v_ext),
                              ("oT", oT)):
                    dmp = nc.dram_tensor(f"dump_{nm}", list(t.shape), cdt,
                                         kind="ExternalOutput")
                    nc.sync.dma_start(out=dmp.ap(), in_=t)

    nc.compile()
    return nc


def _get_nc():
    if "nc" not in _CACHE:
        _CACHE["nc"] = _build()
    return _CACHE["nc"]


def _make_mask():
    """mask[kl, pi, j*512+ql] = 1.0 iff kl + (128*(2*pi+j)) <= ql   (f32)."""
    kl = np.arange(128)[:, None]
    ql = np.arange(512)[None, :]
    m = np.zeros((128, 2, 1024), np.float32)
    for pi in range(2):
        for j in range(2):
            o = 128 * (2 * pi + j)
            m[:, pi, 512 * j:512 * (j + 1)] = (kl + o <= ql)
    return m


def make_in_maps(x, Wq, bq, Wk, bk, Wv, Wp):
    mask = _make_mask()
    in_maps = []
    for c in range(8):
        b, hg = c // 2, c % 2
        hs = slice(hg * HPC, (hg + 1) * HPC)
        in_maps.append({
            "x": np.ascontiguousarray(x[b]),
            "wq": np.ascontiguousarray(Wq[hs].transpose(1, 0, 2).reshape(D, LCOL)),
            "wk": np.ascontiguousarray(Wk[hs].transpose(1, 0, 2).reshape(D, LCOL)),
            "wv": np.ascontiguousarray(Wv[hs].transpose(1, 0, 2).reshape(D, LCOL)),
            "bq": np.ascontiguousarray(bq[hs].reshape(LCOL)),
            "bk": np.ascontiguousarray(bk[hs].reshape(LCOL)),
            "wp": np.ascontiguousarray(Wp[hg * LCOL:(hg + 1) * LCOL, :]),
            "mask": mask,
        })
    return in_maps


def combine(results, Wp, bv, bp):
    """Unshard: sum the two head-group partials per batch + linear bias terms."""
    add = bp + bv.reshape(D) @ Wp
    out = np.empty((B, S, D), np.float32)
    for b in range(B):
        out[b] = results[2 * b]["out"] + results[2 * b + 1]["out"] + add
    return out


def kernel(x, Wq, bq, Wk, bk, Wv, bv, Wp, bp):
    from concourse.bass_utils import run_bass_kernel_spmd

    x = np.asarray(x, np.float32)
    Wq = np.asarray(Wq, np.float32)
    Wk = np.asarray(Wk, np.float32)
    Wv = np.asarray(Wv, np.float32)
    bq = np.asarray(bq, np.float32)
    bk = np.asarray(bk, np.float32)
    bv = np.asarray(bv, np.float32)
    Wp = np.asarray(Wp, np.float32)
    bp = np.asarray(bp, np.float32)

    nc = _get_nc()
    in_maps = make_in_maps(x, Wq, bq, Wk, bk, Wv, Wp)
    res = run_bass_kernel_spmd(nc, in_maps, core_ids=list(range(8)))
    return combine(res.results, Wp, bv, bp)

